# revision 1
# baseline (speedup 1.0000x reference)
"""Trainium2 Bass kernel for the correlation-softargmax flow module.

Math (per batch b, query pixel q=(y,x)):
  c1 = l2norm_C(feature1), warp = l2norm_C(feature2)
  s[l,q] = <3x3 patch of warp at l, 3x3 patch of c1 at q>    (D = 32*9 = 288)
  p = softmax_l(10*s);  flow = (E_p[ix_l] - x, E_p[iy_l] - y)

Because softmax normalizes, only Z = sum_l exp, Sy = sum_l exp*iy, Sx = sum_l
exp*ix are needed per q (flash-attention style, no [L,L] materialization, and
exp(10*s - 30) needs no running max since |10*s| <= 90 and using a fixed shift
keeps everything in fp32 range).

Sharding: 8 cores = 4 batches x 2 query-row halves. Each core holds the full
K-side image (softmax runs over all 4096 l) and 2048 queries.

On-device layout: C=32 on partitions, zero-padded [32,66,66] images make every
3x3 tap a pure AP shift; taps are packed 4+4+1 into d-major patch tensors of
128/128/32 partitions so the score matmuls run K=128 contractions. float32r
(hw-measured: 12-bit-mantissa rounding) runs the PE at 1 cycle/row vs fp32's 4.
"""

import sys

import numpy as np

sys.path.insert(0, "/opt/trn_rl_repo")

import concourse.bass as bass  # noqa: E402
import concourse.mybir as mybir  # noqa: E402
import concourse.tile as tile  # noqa: E402
from concourse import bacc, bass_utils  # noqa: E402

F32 = mybir.dt.float32
F32R = mybir.dt.float32r
F16 = mybir.dt.float16
BF16 = mybir.dt.bfloat16

B, C, H, W = 4, 32, 64, 64
L = H * W              # 4096 match locations
NQ = L // 2            # queries per core
QROWS = H // 2         # query rows per core
N_CORES = 8
SCALE = 10.0
SHIFT = -30.0          # exp(10*s - 30): |10*s|<=90 so no overflow, and a row's
                       # max 10*s is never < -60 so Z stays far above underflow
EPS = 1e-12
TAPS = [(dy, dx) for dy in range(3) for dx in range(3)]

_NC_CACHE = {}
_LAST_RES = None


def _build_nc():
    nc = bacc.Bacc(None, target_bir_lowering=False)

    f1h = nc.dram_tensor("f1h", [C, QROWS + 2, W], F32, kind="ExternalInput")
    f2 = nc.dram_tensor("f2", [C, H, W], F32, kind="ExternalInput")
    w3 = nc.dram_tensor("w3", [128, 96], F32, kind="ExternalInput")
    yq = nc.dram_tensor("yq", [1, NQ], F32, kind="ExternalInput")
    xq = nc.dram_tensor("xq", [1, NQ], F32, kind="ExternalInput")
    outp = nc.dram_tensor("outp", [2, NQ], F32, kind="ExternalOutput")

    n1 = (QROWS + 2) * W   # 2176 pixels in the f1 halo slab

    with tile.TileContext(nc) as tc:
        with tc.tile_pool(name="big", bufs=1) as big, \
             tc.tile_pool(name="work", bufs=1) as work, \
             tc.tile_pool(name="small", bufs=1) as small, \
             tc.tile_pool(name="pp", bufs=3) as pp, \
             tc.tile_pool(name="epi", bufs=2) as epi, \
             tc.tile_pool(name="nps", bufs=1, space="PSUM") as nps, \
             tc.tile_pool(name="sps", bufs=4, space="PSUM") as sps, \
             tc.tile_pool(name="stps", bufs=2, space="PSUM") as stps:

            # ---- load inputs ----
            raw2 = big.tile([C, L], F32, tag="raw2")
            nc.sync.dma_start(out=raw2, in_=f2[:, :, :].rearrange("c h w -> c (h w)"))
            raw1 = big.tile([C, n1], F32, tag="raw1")
            nc.sync.dma_start(out=raw1, in_=f1h[:, :, :].rearrange("c h w -> c (h w)"))
            w3f = small.tile([128, 96], F32, tag="w3f")
            nc.sync.dma_start(out=w3f, in_=w3[:, :])
            w3r = small.tile([128, 96], BF16, tag="w3r")
            nc.vector.tensor_copy(w3r, w3f)
            xqs = small.tile([1, NQ], F32, tag="xqs")
            nc.sync.dma_start(out=xqs, in_=xq[:, :])
            yqs = small.tile([1, NQ], F32, tag="yqs")
            nc.sync.dma_start(out=yqs, in_=yq[:, :])

            onesf = small.tile([C, 1], F32, tag="onesf")
            nc.vector.memset(onesf, 1.0)
            ones32 = small.tile([C, 1], F32R, tag="ones32")
            nc.vector.tensor_copy(ones32, onesf)
            onesbf = small.tile([1, C], F32, tag="onesbf")
            nc.vector.memset(onesbf, 1.0)
            onesb = small.tile([1, C], F32R, tag="onesb")
            nc.vector.tensor_copy(onesb, onesbf)
            shiftc = small.tile([128, 1], F32, tag="shiftc")
            nc.vector.memset(shiftc, SHIFT)
            eps2c = small.tile([1, 1], F32, tag="eps2c")
            nc.vector.memset(eps2c, EPS * EPS)

            # ---- l2 normalization over C (C sits on partitions, so the
            # per-pixel sum of squares comes from a ones-vector matmul; the
            # 1/norm row is broadcast back across partitions with a K=1
            # ones-matmul and the scaling multiply reads it from PSUM) ----
            def normalize(raw, npix, pad, row0, img):
                nchunks = (npix + 511) // 512
                raw3 = raw.rearrange("c (h w) -> c h w", w=W)
                nc.vector.memset(pad, 0.0)
                nrow = work.tile([1, npix], F32, tag=f"nrow{img}",
                                 name=f"nrow{img}")
                for j in range(nchunks):
                    n = min(512, npix - 512 * j)
                    sqc = work.tile([C, 512], F32R, tag="sqc", name="sqc", bufs=2)
                    eng = nc.vector if j % 2 == 0 else nc.gpsimd
                    eng.tensor_mul(sqc[:, :n], raw[:, 512 * j:512 * j + n],
                                   raw[:, 512 * j:512 * j + n])
                    ssp = nps.tile([1, 512], F32, tag="ssp", name="ssp")
                    nc.tensor.matmul(ssp[:, :n], ones32, sqc[:, :n],
                                     start=True, stop=True)
                    # norm = sqrt(ss + eps^2) == max(sqrt(ss), eps) in fp32 here
                    nc.scalar.activation(nrow[:, 512 * j:512 * j + n],
                                         ssp[:, :n],
                                         mybir.ActivationFunctionType.Sqrt,
                                         bias=eps2c)
                # reciprocal is ~6.5 ns/elem per partition lane: bounce the row
                # through a [128, npix/128] layout so it runs 128-wide
                nT = work.tile([128, npix // 128], F32, tag=f"nT{img}",
                               name=f"nT{img}")
                nc.sync.dma_start(
                    out=nT, in_=nrow.rearrange("a (p c) -> a p c", p=128))
                rT = work.tile([128, npix // 128], F32R, tag=f"rT{img}",
                               name=f"rT{img}")
                with nc.allow_low_precision(reason="f32r 1/norm, 12 bits"):
                    nc.vector.reciprocal(rT, nT)
                rrow = work.tile([1, npix], F32R, tag=f"rrow{img}",
                                 name=f"rrow{img}")
                nc.sync.dma_start(
                    out=rrow.rearrange("a (p c) -> a p c", p=128), in_=rT)
                for j in range(nchunks):
                    n = min(512, npix - 512 * j)
                    rows = n // W  # chunks are whole image rows (512 = 8*64)
                    rb = nps.tile([C, 512], F32, tag="rb", name="rb")
                    nc.tensor.matmul(rb[:, :n], onesb,
                                     rrow[:, 512 * j:512 * j + n],
                                     start=True, stop=True)
                    r0 = 8 * j
                    nc.vector.tensor_mul(  # gpsimd cannot read PSUM
                        pad[:, row0 + r0:row0 + r0 + rows, 1:W + 1],
                        raw3[:, r0:r0 + rows, :],
                        rb[:, :n].rearrange("c (h w) -> c h w", w=W),
                    )

            pad2 = big.tile([C, H + 2, W + 2], F16, tag="pad2")
            normalize(raw2, L, pad2, row0=1, img=2)
            pad1 = big.tile([C, QROWS + 2, W + 2], F16, tag="pad1")
            normalize(raw1, n1, pad1, row0=0, img=1)

            # ---- d-major patch tensors: 3 groups of 3 taps (96 partitions);
            # each tap is one strided fp16 DMA copy out of the padded image,
            # so no compute engine touches the patch build ----
            kp3 = [big.tile([96, H, W], F16, tag=f"kp{g}", name=f"kp{g}")
                   for g in range(3)]
            qp3 = [big.tile([96, QROWS, W], F16, tag=f"qp{g}", name=f"qp{g}")
                   for g in range(3)]
            dma_engs = [nc.sync, nc.scalar, nc.gpsimd]
            for t, (dy, dx) in enumerate(TAPS):
                g, j = divmod(t, 3)
                e0 = dma_engs[(2 * t) % len(dma_engs)]
                e1 = dma_engs[(2 * t + 1) % len(dma_engs)]
                e0.dma_start(out=kp3[g][32 * j:32 * j + 32, :, :],
                             in_=pad2[:, dy:dy + H, dx:dx + W])
                e1.dma_start(out=qp3[g][32 * j:32 * j + 32, :, :],
                             in_=pad1[:, dy:dy + QROWS, dx:dx + W])

            # ---- main loop: scores -> exp -> stats, flash-attention style ----
            n_lt = L // 128
            n_qt = NQ // 512
            for qt in range(n_qt):
                stats = stps.tile([3, 512], F32, tag="stats")
                # software-pipelined by one lt: the stats matmul for lt is
                # emitted after the score matmuls of lt+1, so the in-order PE
                # stream never stalls waiting for exp (a stalled PE re-arms
                # the HAM throttle and halves the clock)
                pend = None
                for lt in range(n_lt):
                    s_ps = sps.tile([128, 512], F32, tag="s")
                    for g in range(3):
                        nc.tensor.matmul(
                            s_ps,
                            kp3[g][:, 2 * lt:2 * lt + 2, :],
                            qp3[g][:, 8 * qt:8 * qt + 8, :],
                            start=(g == 0), stop=(g == 2),
                        )
                    if pend is not None:
                        nc.tensor.matmul(stats, w3r[:, 3 * (lt - 1):3 * lt],
                                         pend, start=(lt == 1), stop=False)
                    p_sb = pp.tile([128, 512], BF16, tag="p")
                    nc.scalar.activation(p_sb, s_ps,
                                         mybir.ActivationFunctionType.Exp,
                                         bias=shiftc, scale=SCALE)
                    pend = p_sb
                nc.tensor.matmul(stats, w3r[:, 3 * (n_lt - 1):3 * n_lt], pend,
                                 start=False, stop=True)

                # flow = S/Z - coord. Engines can only address partitions at
                # 0/32/64 bases, so DMA the [3,512] stats onto partition 0.
                st3 = epi.tile([3, 512], F32, tag="st3")
                nc.scalar.copy(st3, stats)
                zsb = epi.tile([1, 3 * 512], F32, tag="zsb")
                nc.sync.dma_start(out=zsb.rearrange("a (b c) -> a b c", c=512),
                                  in_=st3)
                z, sy, sx = zsb[:, 0:512], zsb[:, 512:1024], zsb[:, 1024:1536]
                rz = epi.tile([1, 512], F32, tag="rz")
                nc.vector.reciprocal(rz, z)
                fw = epi.tile([1, 512], F32, tag="fw")
                nc.vector.tensor_mul(fw, sx, rz)
                nc.vector.tensor_sub(fw, fw, xqs[:, 512 * qt:512 * qt + 512])
                fh = epi.tile([1, 512], F32, tag="fh")
                nc.vector.tensor_mul(fh, sy, rz)
                nc.vector.tensor_sub(fh, fh, yqs[:, 512 * qt:512 * qt + 512])
                nc.sync.dma_start(out=outp[0:1, 512 * qt:512 * qt + 512], in_=fw)
                nc.sync.dma_start(out=outp[1:2, 512 * qt:512 * qt + 512], in_=fh)

    nc.finalize()
    return nc


def _host_consts():
    p = np.arange(128)
    w3 = np.zeros((128, 96), np.float32)
    for t in range(32):
        w3[:, 3 * t] = 1.0
        w3[:, 3 * t + 1] = 2 * t + p // 64   # global iy of l = 128*lt + p
        w3[:, 3 * t + 2] = p % 64            # global ix
    q = np.arange(NQ)
    xq = (q % W).astype(np.float32)[None]
    ly = (q // W).astype(np.float32)
    return w3, xq, ly


def kernel(feature1, feature2):
    feature1 = np.ascontiguousarray(feature1, np.float32)
    feature2 = np.ascontiguousarray(feature2, np.float32)
    w3, xq, ly = _host_consts()

    f1p = np.zeros((B, C, H + 2, W), np.float32)
    f1p[:, :, 1:H + 1, :] = feature1

    in_maps = []
    for core in range(N_CORES):
        b, h = divmod(core, 2)
        in_maps.append({
            "f1h": np.ascontiguousarray(f1p[b, :, h * QROWS:h * QROWS + QROWS + 2, :]),
            "f2": np.ascontiguousarray(feature2[b]),
            "w3": w3,
            "yq": (ly + h * QROWS)[None].astype(np.float32),
            "xq": xq,
        })

    if "nc" not in _NC_CACHE:
        _NC_CACHE["nc"] = _build_nc()
    res = bass_utils.run_bass_kernel_spmd(
        _NC_CACHE["nc"], in_maps, core_ids=list(range(N_CORES)))
    global _LAST_RES
    _LAST_RES = res

    out = np.zeros((B, 2, H, W), np.float32)
    for core in range(N_CORES):
        b, h = divmod(core, 2)
        out[b, :, h * QROWS:(h + 1) * QROWS, :] = (
            res.results[core]["outp"].reshape(2, QROWS, W))
    return out

